# revision 8
# baseline (speedup 1.0000x reference)
"""Trainium2 Bass kernel v5 for the AttentionBlock, data-parallel over batch
across 8 cores.  Per-core problem (one batch element):

  x [512, 4096] -> qkv (1x1 conv) -> full 4096x4096 spatial attention
  -> out-proj + residual -> y [512, 4096]

v3 structure:
  - scores: fp8e4 DoubleRow matmuls, 4-way row-tiled. k/q are stored
    unscaled in fp8 as [32, 2, *] (hid halves interleaved); four key tiles
    compute concurrently in the PE array (512 cycles for 4 tiles x 512
    queries).  The 1/sqrt(hid) scale rides the exp's free scale operand.
  - exp: split between ScalarE (exact exp(S/8) -> fp8e4) and DVE
    (Schraudolph: int8(S*A/8 + 56.5) bitcast fp8e4), c2-granular
    [128,512] ops so PSUM stays within 8 banks with double buffering.
  - P*V: fp8e4 DoubleRow, two key tiles per instruction, 65th ones-column
    accumulates the softmax denominators.
  - normalization via DMA-broadcast reciprocal (DRAM scratch reshape).
  - out-proj (K=64 bf16) + residual skip accumulated in PSUM via an
    f32r identity-matmul on x; b_out rides the ScalarE PSUM->SBUF copy.
  - x stays f32 in SBUF and feeds matmuls bitcast to float32r (full PE
    rate), so its DMA avoids the casting (gpsimd) queue.
"""

import numpy as np
import ml_dtypes

from concourse import bacc, tile, mybir
from concourse import bass_utils
from concourse.bass import ds, ts
from concourse.masks import make_identity

F32 = mybir.dt.float32
F32R = mybir.dt.float32r
BF16 = mybir.dt.bfloat16
FP8 = mybir.dt.float8e4
I8 = mybir.dt.int8
EXP = mybir.ActivationFunctionType.Exp
IDENT = mybir.ActivationFunctionType.Identity
COPY = mybir.ActivationFunctionType.Copy
DR = mybir.MatmulPerfMode.DoubleRow
MULT = mybir.AluOpType.mult
ADD = mybir.AluOpType.add

B = 8
C = 512
HID = 64
N = 4096
NB = 1024            # query block (4 blocks)
NPAIR = 16           # key-tile pairs
NG = 8               # groups of 4 key tiles
SCALE = 0.125        # 1/sqrt(64), applied at exp time
A8 = float(8.0 / np.log(2.0)) * SCALE
B8 = 56.5
NFILL = 3            # zero-weight PE filler matmuls per attention slot

# slots (g, c2_idx) whose ODD pair-exp moves from the DVE to ScalarE (balance)
ACT_EXTRA = {(2, 0), (5, 1)}


def build_bass(stage=4, ndev=B):
    nc = bacc.Bacc(
        "TRN2",
        target_bir_lowering=False,
        debug=False,
        enable_asserts=False,
        num_devices=ndev,
    )
    x = nc.dram_tensor("x", [C, N], F32R, kind="ExternalInput").ap()
    wiA = nc.dram_tensor("wiA", [C, 128], F32R, kind="ExternalInput").ap()  # [q|v]
    wiB = nc.dram_tensor("wiB", [C, HID], F32R, kind="ExternalInput").ap()  # [k]
    biasA = nc.dram_tensor("biasA", [128, 1], F32, kind="ExternalInput").ap()
    biasB = nc.dram_tensor("biasB", [HID, 1], F32, kind="ExternalInput").ap()
    woT = nc.dram_tensor("woT", [HID, C], BF16, kind="ExternalInput").ap()
    bout = nc.dram_tensor("bout", [128, 4], F32, kind="ExternalInput").ap()
    y = nc.dram_tensor("y", [C, N], F32, kind="ExternalOutput").ap()
    scr_d = nc.dram_tensor("scr_d", [4, NB], F32, kind="Internal").ap()
    scr_r = nc.dram_tensor("scr_r", [4, NB], F32, kind="Internal").ap()

    xr = x.rearrange("(a p) n -> p a n", p=128)   # [128, 4, N]
    yr = y.rearrange("(a p) n -> p a n", p=128)

    with tile.TileContext(nc) as tc:
        with (
            nc.allow_low_precision(reason="bf16/fp8 attention math is intended"),
            tc.tile_pool(name="const", bufs=1) as cpool,
            tc.tile_pool(name="xin", bufs=1) as xpool,
            tc.tile_pool(name="big", bufs=1) as bigpool,
            tc.tile_pool(name="u", bufs=3) as upool,
            tc.tile_pool(name="bc", bufs=2) as bcpool,
            tc.tile_pool(name="yout", bufs=3) as ypool,
            tc.tile_pool(name="psum", bufs=1, space="PSUM") as pp,
            tc.tile_pool(name="psum2", bufs=2, space="PSUM") as pp2,
        ):
            # ---- constants ----
            idf = cpool.tile([128, 128], F32)
            make_identity(nc, idf[:, :])
            bA = cpool.tile([128, 1], F32)
            nc.sync.dma_start(bA[:, :], biasA)
            bB = cpool.tile([HID, 1], F32)
            nc.sync.dma_start(bB[:, :], biasB)
            bo = cpool.tile([128, 4], F32)
            nc.sync.dma_start(bo[:, :], bout)
            wA = cpool.tile([128, 4, 128], F32R)
            nc.sync.dma_start(wA[:, :, :], wiA.rearrange("(a p) m -> p a m", p=128))
            wB = cpool.tile([128, 4, HID], F32R)
            nc.sync.dma_start(wB[:, :, :], wiB.rearrange("(a p) m -> p a m", p=128))
            wo = cpool.tile([HID, C], BF16)
            nc.sync.dma_start(wo[:, :], woT)

            # ---- persistent tensors ----
            x_sb = xpool.tile([128, 4, N], F32R)         # x as f32r (full-rate matmul rhs)
            qv = bigpool.tile([128, N], F32)             # q rows 0:64, v rows 64:128
            k_sb = bigpool.tile([HID, N], F32)
            q8 = bigpool.tile([128, 2, N], FP8)          # 4x replicated, hid-split
            ks8 = bigpool.tile([128, NG, 2, 128], FP8)   # 4-way tiled key tiles
            vt2 = bigpool.tile([128, NPAIR, 2, 80], FP8)  # v^T pairs + ones col @64
            O = bigpool.tile([HID, N], BF16)             # normalized attention out
            nc.gpsimd.memset(vt2[:, :, :, :], 0.0)
            nc.gpsimd.memset(vt2[:, :, :, 64:65], 1.0)
            z8 = cpool.tile([128, 2, 80], FP8)
            nc.gpsimd.memset(z8[:, :, :], 0.0)


            # ---- phase 1+2: qkv projection, fp8 assembly, v-transpose ----
            for nq in range(N // NB):
                nblk = ds(nq * NB, NB)
                for kc in range(4):
                    nc.sync.dma_start(x_sb[:, kc, nblk], xr[:, kc, nblk])
                psA = pp.tile([128, NB], F32, tag="se", name=f"psA_{nq}")
                psB = pp.tile([HID, NB], F32, tag="so", name=f"psB_{nq}")
                for c2 in range(0, NB, 512):
                    cblk = ds(nq * NB + c2, 512)
                    for kc in range(4):
                        nc.tensor.matmul(
                            psA[:, c2:c2 + 512], wA[:, kc, :], x_sb[:, kc, cblk],
                            start=(kc == 0), stop=(kc == 3),
                        )
                    for kc in range(4):
                        nc.tensor.matmul(
                            psB[:, c2:c2 + 512], wB[:, kc, :], x_sb[:, kc, cblk],
                            start=(kc == 0), stop=(kc == 3),
                        )
                nc.scalar.activation(qv[:, nblk], psA[:, :], IDENT,
                                     bias=bA[:, 0:1])
                nc.vector.tensor_scalar_add(k_sb[:, nblk], psB[:, :],
                                            bB[:, 0:1])
                # v-transposes for this block (v = qv rows 64:128)
                ps_t = pp2.tile([128, NB], F32, tag="o", name=f"pst_{nq}")
                for l in range(8):
                    nc.tensor.transpose(
                        ps_t[:, ds(l * 64, 64)],
                        qv[64:128, ds(nq * NB + l * 128, 128)],
                        idf[64:128, 64:128],
                    )
                pt_v = ps_t[:, 0:512].rearrange("p (i e c) -> p i e c", i=4, e=2)
                nc.vector.tensor_copy(
                    vt2[:, ds(4 * nq, 4), :, 0:64], pt_v[:, :, :, :])
                # fp8 q/k assembly for this block (DMA-cast on gpsimd queue)
                for i in range(4):
                    for j in range(2):
                        nc.gpsimd.dma_start(
                            q8[ds(32 * i, 32), j, nblk], qv[ds(32 * j, 32), nblk])
                        nc.gpsimd.dma_start(
                            ks8[ds(32 * i, 32), ds(2 * nq, 2), j, :],
                            k_sb[ds(32 * j, 32), nblk]
                            .rearrange("p (g f) -> p g f", f=512)
                            [:, :, ds(i * 128, 128)],
                        )

            if stage == 1:
                nc.sync.dma_start(yr[:, 0, :], qv[:, :])
                nc.sync.dma_start(yr[0:HID, 1, :], k_sb[:, :])
                q8f = bigpool.tile([128, 2 * N], F32)
                nc.vector.tensor_copy(q8f[:, :], q8.rearrange("p a n -> p (a n)"))
                nc.sync.dma_start(yr[:, 2, :], q8f[:, 0:N])
                nc.sync.dma_start(yr[:, 3, :], q8f[:, N:2 * N])

            xv = x_sb[:, :, :].bitcast(F32)

            def emit_outproj(hh):
                for oc in range(4):
                    psY = pp2.tile([128, NB], F32, tag="o", name=f"psY_{hh}_{oc}")
                    for ci in range(2):
                        oblk = ds(hh * NB + ci * 512, 512)
                        nc.tensor.matmul(
                            psY[:, ds(ci * 512, 512)], wo[:, ts(oc, 128)],
                            O[:, oblk], start=True, stop=True,
                        )
                    y_sb = ypool.tile([128, NB], F32, tag="yt",
                                      name=f"y_{hh}_{oc}")
                    nc.vector.scalar_tensor_tensor(
                        y_sb[:, :], psY[:, :], bo[:, oc:oc + 1],
                        xv[:, oc, ds(hh * NB, NB)], ADD, ADD)
                    nc.sync.dma_start(yr[:, oc, ds(hh * NB, NB)], y_sb[:, :])

            # ---- phase 3: attention ----
            for h in range(N // NB if stage >= 2 else 0):
                hblk = ds(h * NB, NB)
                psO = pp2.tile([128, NB], F32, tag="o", name=f"psO_{h}")
                u4s = {}
                for g in range(NG):
                    for ci in range(2):
                        qblk = ds(h * NB + ci * 512, 512)
                        se = pp.tile([128, NB], F32, tag="se",
                                     name=f"se_{h}_{g}_{ci}")
                        so = pp.tile([128, NB], F32, tag="so",
                                     name=f"so_{h}_{g}_{ci}")
                        for i in range(4):
                            dst = se if i < 2 else so
                            nc.tensor.matmul(
                                dst[:, ds((i % 2) * 512, 512)],
                                ks8[ds(32 * i, 32), g, :, :],
                                q8[ds(32 * i, 32), :, qblk],
                                start=True, stop=True,
                                tile_position=(32 * i, 0),
                                perf_mode=DR,
                            )
                        u4 = upool.tile([128, 2, 2, 512], FP8, tag="u4",
                                        name=f"u4_{h}_{g}_{ci}")
                        u4i = u4.bitcast(I8)
                        nc.scalar.activation(
                            u4[:, 0, :, :], se[:, :], EXP, scale=SCALE)
                        if (g, ci) in ACT_EXTRA:
                            nc.scalar.activation(
                                u4[:, 1, :, :], so[:, :], EXP, scale=SCALE)
                        else:
                            nc.vector.tensor_scalar(
                                u4i[:, 1, :, :], so[:, :], A8, B8, MULT, ADD)
                        u4s[ci] = u4
                    # P*V pi-major so each vt2 weight loads once per group
                    for pi in range(2):
                        for ci in range(2):
                            nc.tensor.matmul(
                                psO[0:65, ds(ci * 512, 512)],
                                vt2[:, 2 * g + pi, :, 0:65], u4s[ci][:, pi, :, :],
                                start=(g == 0 and pi == 0),
                                stop=(g == NG - 1 and pi == 1),
                                perf_mode=DR,
                            )
                    # zero-weight fillers keep the PE array busy so the HAM
                    # clock gate stays at K=8/8 through the exp-bound phase
                    if g < NG - 1:
                        for f in range(NFILL):
                            nc.tensor.matmul(
                                psO[0:65, ds((f % 2) * 512, 512)],
                                z8[:, :, 0:65],
                                q8[:, :, ds(((g * NFILL + f) % 7) * 512, 512)],
                                start=False, stop=False,
                                perf_mode=DR,
                            )
                if stage == 2:
                    po_sb = bcpool.tile([80, NB], F32, tag="dbg", name=f"dbg_{h}")
                    nc.vector.tensor_copy(po_sb[:, :], psO[:, :])
                    nc.sync.dma_start(yr[0:80, h, :NB], po_sb[:, :])
                    continue
                # softmax denominators -> reciprocal -> DMA broadcast
                dsb = bcpool.tile([1, NB], F32, tag="d", name=f"d_{h}")
                nc.scalar.activation(dsb[:, :], psO[64:65, :], COPY)
                nc.sync.dma_start(scr_d[h:h + 1, :], dsb[:, :])
                dcol = bcpool.tile([128, 8], F32, tag="dc", name=f"dc_{h}")
                nc.sync.dma_start(
                    dcol[:, :], scr_d[h:h + 1, :].rearrange("o (p f) -> (o p) f", p=128)
                )
                rcol = bcpool.tile([128, 8], F32, tag="rc", name=f"rc_{h}")
                nc.vector.reciprocal(rcol[:, :], dcol[:, :])
                nc.sync.dma_start(
                    scr_r[h:h + 1, :].rearrange("o (p f) -> (o p) f", p=128), rcol[:, :]
                )
                bc = bcpool.tile([HID, NB], F32, tag="bc", name=f"bc_{h}")
                nc.gpsimd.dma_start(bc[:, :], scr_r[h:h + 1, :].to_broadcast([HID, NB]))
                nc.vector.tensor_mul(O[:, hblk], psO[0:HID, :], bc[:, :])

                # ---- phase 5 (deferred one block): out-proj + skip + bias ----
                if h > 0:
                    emit_outproj(h - 1)
            if stage >= 3:
                emit_outproj(3)

    nc.compile()
    return nc


_NC = None


def _get_nc():
    global _NC
    if _NC is None:
        _NC = build_bass()
    return _NC


def make_in_maps(x, w_in, b_in, w_out, b_out):
    w_in = np.asarray(w_in, np.float32)
    b_in = np.asarray(b_in, np.float32)
    wq = w_in[0:HID]
    wk = w_in[HID:2 * HID]
    wv = w_in[2 * HID:3 * HID]
    wiA = np.ascontiguousarray(np.concatenate([wq, wv], 0).T, np.float32)
    wiB = np.ascontiguousarray(wk.T, np.float32)
    biasA = np.ascontiguousarray(
        np.concatenate([b_in[0:HID], b_in[2 * HID:3 * HID]]).reshape(128, 1),
        np.float32)
    biasB = np.ascontiguousarray(b_in[HID:2 * HID].reshape(HID, 1), np.float32)
    woTn = np.ascontiguousarray(
        np.asarray(w_out, np.float32).T.astype(ml_dtypes.bfloat16))
    boutn = np.ascontiguousarray(
        np.asarray(b_out, np.float32).reshape(4, 128).T, np.float32)
    x = np.asarray(x, np.float32)
    return [
        {
            "x": np.ascontiguousarray(x[b].reshape(C, N)),
            "wiA": wiA, "wiB": wiB, "biasA": biasA, "biasB": biasB,
            "woT": woTn, "bout": boutn,
        }
        for b in range(B)
    ]


def kernel(x, w_in, b_in, w_out, b_out):
    nc = _get_nc()
    in_maps = make_in_maps(x, w_in, b_in, w_out, b_out)
    res = bass_utils.run_bass_kernel_spmd(nc, in_maps, core_ids=list(range(B)))
    H = int(np.sqrt(N))
    out = np.stack([np.asarray(res.results[b]["y"]).reshape(C, H, H) for b in range(B)])
    return out.astype(np.float32)
